# revision 1
# baseline (speedup 1.0000x reference)
"""Causal self-attention Trainium2 Bass kernel.

Problem: B=4, T=2048, C=1024, NH=16, HD=64, fp32.
Sharding: 2D over 8 cores = 4 batches x 2 head-groups (8 heads each).
Each core computes, for its (batch b, head-group g):
    q/k/v = x[b] @ W{q,k,v}[rows_g].T + b{q,k,v}[rows_g]
    causal attention over its 8 heads
    partial_out = y_local @ Wp[:, cols_g].T        (host adds the two
    group partials per batch plus bp).

Schedule (per core): the four 512-wide query slices are processed as
one software-pipelined loop — projections for slice n (PE-heavy, fp16
weights/x), then causal attention for i-tile n (ACT-heavy exp), then
the output projection for those rows — so the Tile scheduler can fill
each engine's stalls with the neighbouring stage's work.

Layouts:
    kT      : [128, T] per head-pair (head-dim on partitions, 2 heads
              stacked 64+64), persistent; qT is a transient [128, 512]
              per-pair tile for the current i-slice.
    S^T     : two K=64 row-tiled fp16 matmuls (heads concurrent in the
              PE array) into one 2-bank PSUM tile [j=128, cols h0|h1].
    v_ext   : [128, 130] per (j-tile, pair): [v_h0 | 1 | v_h1 | 1]; the
              ones column makes the y-matmul (M=65) also emit the
              softmax denominator as PSUM row 64.
    softmax : no max-subtraction (scores are O(3)); exp on ACT; causal
              masking by a DVE multiply with triangle masks on diagonal
              blocks; P is kept in fp16 for the fast PE weight-load path.
    y^T     : [65, 512] PSUM per head; normalized on eviction using
              reciprocal of row 64 broadcast via gpsimd.

All matmuls run in fp16 (weights/x pre-cast on host; q/k/v/P/y are
small-range, and all accumulation is fp32 in PSUM, so total cost is
~3e-4 rel err) — fp16 gets the fast FWL weight-load path and full rate
at any moving width.
Built with bacc.Bacc + compile() so multi-wait instructions are
legalized (walrus allows one sync-wait per engine instruction); PSUM
slot-recycle deps are pre-absorbed into dummy LDWEIGHTS ops so fused
fp32r LDW+MM structs keep a single wait.
"""

import numpy as np
import ml_dtypes

import concourse.bass as bass
import concourse.mybir as mybir
import concourse.tile as tile
from concourse import bacc
from concourse.tile_rust import add_dep_helper

B, T, C = 4, 2048, 1024
NH, HD = 16, 64
HPG = 8            # heads per group (per core)
NPAIR = HPG // 2   # head pairs per core
CL = HPG * HD      # 512 local channels
F32 = mybir.dt.float32
F32R = mybir.dt.float32r
BF16 = mybir.dt.bfloat16
FP16 = mybir.dt.float16
EXP = mybir.ActivationFunctionType.Exp
SCALE = 1.0 / np.sqrt(HD)
N_CORES = 8
MOFF = (0, 512, 896, 1152)     # packed mask offsets, widths 512/384/256/128


def attention_body(tc, outs, ins, t=T):
    nc = tc.nc
    nit = t // 512            # i-tiles (queries) == x slices
    njb = t // 128            # j-blocks (keys)
    nkt = C // 128            # contraction tiles for projections

    xT = ins["xT"]            # [C, t] bf16
    wqT, wkT, wvT = ins["wqT"], ins["wkT"], ins["wvT"]   # [C, CL] bf16
    wpT = ins["wpT"]          # [CL, C] f32
    bq, bk = ins["bq"], ins["bk"]      # [128, NPAIR] f32
    bvt = ins["bvt"]          # [128, CL] f32 (bv tiled across partitions)
    masks = ins["masks"]      # [128, 1408] packed diagonal masks
    out = outs["out"]         # [t, C] f32

    dum = {}

    def _absorb(deps, first_mms):
        """Absorb multi-lane PSUM slot-recycle deps into dummy LDWEIGHTS
        ops (one per dep) so the group's first matmul keeps at most one
        sync-wait (the fused fp32r LDW+MM struct allows only one; the
        wait-elision pass only credits real engine instructions)."""
        deps = [d for d in deps if d is not None]
        for d in deps:
            ld = nc.tensor.ldweights(weights=dum["t"][0:1, 0:1])
            add_dep_helper(ld.ins, d.ins, reason="absorb slot release")
            for mm in first_mms:
                add_dep_helper(mm.ins, ld.ins, sync=False,
                               reason="order after absorber")

    with tc.tile_pool(name="consts", bufs=1) as consts:
        dum["t"] = consts.tile([1, 2], BF16, tag="dum", name="dum")
        nc.vector.memset(dum["t"], 0)
        nc.tensor.ldweights(weights=dum["t"][0:1, 0:1])  # prime dum dep
        mks = consts.tile([128, 1408], FP16, tag="mks", name="mks")
        nc.sync.dma_start(out=mks, in_=masks)
        bq_t = consts.tile([128, NPAIR], F32, tag="bq", name="bq_t")
        nc.sync.dma_start(out=bq_t, in_=bq)
        bk_t = consts.tile([128, NPAIR], F32, tag="bk", name="bk_t")
        nc.sync.dma_start(out=bk_t, in_=bk)
        bvt_t = consts.tile([128, CL], F32, tag="bvt", name="bvt_t")
        nc.sync.dma_start(out=bvt_t, in_=bvt)

        with tc.tile_pool(name="persist", bufs=1) as pers, \
             tc.tile_pool(name="wts", bufs=1) as wts, \
             tc.tile_pool(name="xin", bufs=2) as xin, \
             tc.tile_pool(name="qy", bufs=2) as qy, \
             tc.tile_pool(name="ptp", bufs=3) as ptp, \
             tc.tile_pool(name="sm", bufs=2) as sm, \
             tc.tile_pool(name="ps1", bufs=1, space="PSUM") as ps1, \
             tc.tile_pool(name="psS", bufs=2, space="PSUM") as psS, \
             tc.tile_pool(name="psY", bufs=2, space="PSUM") as psY, \
             tc.tile_pool(name="psO", bufs=1, space="PSUM") as psO:
            kT = [pers.tile([128, t], FP16, tag=f"kT{p}", name=f"kT{p}")
                  for p in range(NPAIR)]
            vext = [pers.tile([128, njb * 130], FP16, tag=f"vext{p}",
                              name=f"vext{p}") for p in range(NPAIR)]
            for p in range(NPAIR):
                ones_view = vext[p][:, :].rearrange(
                    "q (jt two d) -> q jt two d", jt=njb, two=2)[:, :, :, 64:65]
                nc.vector.memset(ones_view, 1.0)

            wq_t, wk_t, wv_t = [], [], []
            for kk in range(nkt):
                for lst, wsrc, tg in ((wq_t, wqT, "wq"), (wk_t, wkT, "wk"),
                                      (wv_t, wvT, "wv")):
                    w = wts.tile([128, CL], FP16, tag=f"{tg}{kk}",
                                 name=f"{tg}{kk}")
                    nc.sync.dma_start(
                        out=w, in_=wsrc[128 * kk:128 * (kk + 1), :])
                    lst.append(w)
            wp_t = []
            for p in range(NPAIR):
                w = wts.tile([128, C], FP16, tag=f"wp{p}", name=f"wp{p}")
                nc.sync.dma_start(
                    out=w, in_=wpT[128 * p:128 * (p + 1), :])
                wp_t.append(w)

            ps1_hist = []   # (evictor, last mm) per ps1 slot (bufs=2)
            psS_hist = []   # ([readers], last S mm) per psS slot (bufs=2)
            psY_hist = []   # ([norm insts], [last y mms]) per group
            psO_hist = []   # (ot copy, last out mm) per psO slot (bufs=1)

            for n in range(nit):
                # ---- projections for slice n (bf16) ----
                xts = []
                for kk in range(nkt):
                    xt = xin.tile([128, 512], FP16, tag=f"x{kk}",
                                  name=f"x{kk}")
                    nc.sync.dma_start(
                        out=xt,
                        in_=xT[128 * kk:128 * (kk + 1),
                               512 * n:512 * (n + 1)])
                    xts.append(xt)

                def group(body_mms, evict_fn, hist=ps1_hist, dist=1):
                    k = len(hist)
                    prev = hist[k - dist] if k >= dist else None
                    mms = body_mms()
                    if prev is not None:
                        _absorb([prev[0], prev[1]], [mms[0]])
                    ev = evict_fn()
                    hist.append((ev, mms[-1]))

                qTs = []
                for p in range(NPAIR):
                    qp = qy.tile([128, 512], FP16, tag=f"qT{p}",
                                 name=f"qT{p}")
                    qTs.append(qp)
                for wt, bt, dsts in ((wq_t, bq_t, "q"), (wk_t, bk_t, "k")):
                    for p in range(NPAIR):
                        ps = ps1.tile([128, 512], F32, tag="ps1",
                                      name="ps1q")

                        def mk(ps=ps, wt=wt, p=p):
                            return [nc.tensor.matmul(
                                ps,
                                lhsT=wt[kk][:, 128 * p:128 * (p + 1)],
                                rhs=xts[kk],
                                start=(kk == 0), stop=(kk == nkt - 1))
                                for kk in range(nkt)]

                        if dsts == "q":
                            def ev(ps=ps, bt=bt, p=p):
                                return nc.vector.tensor_scalar_add(
                                    out=qTs[p], in0=ps,
                                    scalar1=bt[:, p:p + 1])
                        else:
                            def ev(ps=ps, bt=bt, p=p, n=n):
                                return nc.vector.tensor_scalar_add(
                                    out=kT[p][:, 512 * n:512 * (n + 1)],
                                    in0=ps, scalar1=bt[:, p:p + 1])
                        group(mk, ev)
                for tb in range(4):
                    jt = 4 * n + tb
                    ps = ps1.tile([128, CL], F32, tag="ps1", name="ps1v")

                    def mk(ps=ps, tb=tb):
                        return [nc.tensor.matmul(
                            ps,
                            lhsT=xts[kk][:, 128 * tb:128 * (tb + 1)],
                            rhs=wv_t[kk],
                            start=(kk == 0), stop=(kk == nkt - 1))
                            for kk in range(nkt)]

                    def ev(ps=ps, jt=jt):
                        last = None
                        for p in range(NPAIR):
                            dst = vext[p][:, 130 * jt:130 * (jt + 1)
                                          ].rearrange(
                                "q (two d) -> q two d", two=2)[:, :, 0:64]
                            last = nc.vector.tensor_add(
                                out=dst,
                                in0=ps[:, 128 * p:128 * (p + 1)].rearrange(
                                    "q (two d) -> q two d", two=2),
                                in1=bvt_t[:, 128 * p:128 * (p + 1)
                                          ].rearrange(
                                    "q (two d) -> q two d", two=2))
                        return last

                    group(mk, ev)

                # ---- attention for i-tile n ----
                it = n
                njb_i = 4 * it + 4
                yTs = []
                for p in range(NPAIR):
                    ky = len(psY_hist)
                    prevy = psY_hist[ky - 1] if ky >= 1 else None
                    psy = [psY.tile([65, 512], F32, tag="psY", name="psy")
                           for _ in range(2)]
                    first_ymms, last_ymms, norms = [], [], []
                    for m in range(njb_i):
                        dm = m - 4 * it
                        off = 128 * dm if dm >= 0 else 0
                        w = 512 - off
                        ks = len(psS_hist)
                        prevs = psS_hist[ks - 2] if ks >= 2 else None
                        pss = psS.tile([128, 1024], F32, tag="psS",
                                       name="pss")
                        smms = []
                        for h in range(2):
                            hb = 64 * h
                            smms.append(nc.tensor.matmul(
                                pss[:, 512 * h + off:512 * (h + 1)],
                                lhsT=kT[p][hb:hb + 64,
                                           128 * m:128 * (m + 1)],
                                rhs=qTs[p][hb:hb + 64, off:512],
                                start=True, stop=True))
                        if prevs is not None:
                            _absorb(list(prevs[0]) + [prevs[1]], [smms[0]])
                        pt = ptp.tile([128, 1024], FP16, tag="pt", name="pt")
                        if dm < 0:
                            ex = nc.scalar.activation(
                                out=pt, in_=pss, func=EXP,
                                scale=float(SCALE))
                            rhs = [pt[:, 0:512], pt[:, 512:1024]]
                            psS_hist.append(([ex], smms[-1]))
                        else:
                            pss3 = pss.rearrange(
                                "q (h w) -> q h w", h=2)[:, :, off:512]
                            pt3 = pt.rearrange(
                                "q (h w) -> q h w", h=2)[:, :, off:512]
                            ex = nc.scalar.activation(
                                out=pt3, in_=pss3, func=EXP,
                                scale=float(SCALE))
                            ptm = ptp.tile([128, 2, 512], FP16, tag="ptm",
                                           name="ptm", bufs=2)
                            mi = nc.vector.tensor_mul(
                                out=ptm[:, :, off:512],
                                in0=pt3,
                                in1=mks[:, MOFF[dm]:MOFF[dm] + w
                                        ].unsqueeze(1).broadcast_to(
                                    [128, 2, w]))
                            rhs = [ptm[:, h, off:512] for h in range(2)]
                            psS_hist.append(([ex, mi], smms[-1]))
                        for h in range(2):
                            ymm = nc.tensor.matmul(
                                psy[h][:, off:512],
                                lhsT=vext[p][:, 130 * m + 65 * h:
                                             130 * m + 65 * (h + 1)],
                                rhs=rhs[h],
                                start=(m == 0), stop=(m == njb_i - 1))
                            if m == 0:
                                first_ymms.append(ymm)
                            if m == njb_i - 1:
                                last_ymms.append(ymm)
                    if prevy is not None:
                        _absorb(list(prevy[0]) + list(prevy[1]), first_ymms)
                    yp = qy.tile([128, 512], FP16, tag=f"yT{p}",
                                 name=f"yT{p}")
                    yTs.append(yp)
                    for h in range(2):
                        rc = sm.tile([1, 512], F32, tag="rc", name="rc",
                                     bufs=2)
                        nc.vector.reciprocal(out=rc, in_=psy[h][64:65, :])
                        bc = sm.tile([64, 512], F32, tag="bc", name="bc",
                                     bufs=3)
                        nc.gpsimd.partition_broadcast(
                            out_ap=bc, in_ap=rc, channels=64)
                        norms.append(nc.vector.tensor_mul(
                            out=yp[64 * h:64 * (h + 1), :],
                            in0=psy[h][0:64, :], in1=bc))
                    psY_hist.append((norms, last_ymms))
                # ---- output projection for this i-tile's rows ----
                for tb in range(4):
                    for oh in range(2):
                        ko = len(psO_hist)
                        prevo = psO_hist[ko - 1] if ko >= 1 else None
                        pso = psO.tile([128, 512], F32, tag="psO",
                                       name="pso")
                        omms = [nc.tensor.matmul(
                            pso,
                            lhsT=yTs[p][:, 128 * tb:128 * (tb + 1)],
                            rhs=wp_t[p][:, 512 * oh:512 * (oh + 1)],
                            start=(p == 0), stop=(p == NPAIR - 1))
                            for p in range(NPAIR)]
                        if prevo is not None:
                            _absorb([prevo[0], prevo[1]], [omms[0]])
                        ot = sm.tile([128, 512], F32, tag="ot", name="ot",
                                     bufs=3)
                        oc = nc.vector.tensor_copy(out=ot, in_=pso)
                        nc.sync.dma_start(
                            out=out[512 * n + 128 * tb:
                                    512 * n + 128 * (tb + 1),
                                    512 * oh:512 * (oh + 1)],
                            in_=ot)
                        psO_hist.append((oc, omms[-1]))


def build_nc(t=T):
    nc = bacc.Bacc("TRN2", target_bir_lowering=False, debug=False)
    ins = {
        "xT": nc.dram_tensor("xT", [C, t], FP16, kind="ExternalInput").ap(),
        "wqT": nc.dram_tensor("wqT", [C, CL], FP16,
                              kind="ExternalInput").ap(),
        "wkT": nc.dram_tensor("wkT", [C, CL], FP16,
                              kind="ExternalInput").ap(),
        "wvT": nc.dram_tensor("wvT", [C, CL], FP16,
                              kind="ExternalInput").ap(),
        "wpT": nc.dram_tensor("wpT", [CL, C], FP16, kind="ExternalInput").ap(),
        "bq": nc.dram_tensor("bq", [128, NPAIR], F32,
                             kind="ExternalInput").ap(),
        "bk": nc.dram_tensor("bk", [128, NPAIR], F32,
                             kind="ExternalInput").ap(),
        "bvt": nc.dram_tensor("bvt", [128, CL], F32,
                              kind="ExternalInput").ap(),
        "masks": nc.dram_tensor("masks", [128, 1408], FP16,
                                kind="ExternalInput").ap(),
    }
    outs = {
        "out": nc.dram_tensor("out", [t, C], F32, kind="ExternalOutput").ap(),
    }
    with tile.TileContext(nc) as tc:
        attention_body(tc, outs, ins, t=t)
    nc.compile()
    return nc


def make_masks():
    """Packed multiplicative causal masks for diagonal blocks dm=0..3
    covering computed region [off:512], off=min(128*dm, 256); widths
    512/384/256/256 at offsets MOFF. mask[jj, c] = 1 iff
    jj <= c + off - 128*dm (c relative to off)."""
    mk = np.zeros((128, 1408), np.float16)
    for dm in range(4):
        off = 128 * dm
        w = 512 - off
        cols = np.arange(w)[None, :] + off - 128 * dm
        mk[:, MOFF[dm]:MOFF[dm] + w] = (
            np.arange(128)[:, None] <= cols).astype(np.float16)
    return mk


def make_core_inputs(x, Wq, bq, Wk, bk, Wv, bv, Wp, b, g):
    """Host-side shard + layout prep for core (batch b, head-group g)."""
    rows = slice(CL * g, CL * (g + 1))
    bf = np.float16
    return {
        "xT": np.ascontiguousarray(x[b].T.astype(bf)),
        "wqT": np.ascontiguousarray(Wq[rows, :].T.astype(bf)),
        "wkT": np.ascontiguousarray(Wk[rows, :].T.astype(bf)),
        "wvT": np.ascontiguousarray(Wv[rows, :].T.astype(bf)),
        "wpT": np.ascontiguousarray(Wp[:, rows].T.astype(bf)),
        "bq": np.ascontiguousarray(bq[rows].reshape(NPAIR, 128).T),
        "bk": np.ascontiguousarray(bk[rows].reshape(NPAIR, 128).T),
        "bvt": np.ascontiguousarray(
            np.tile(bv[rows][None, :], (128, 1)).astype(np.float32)),
        "masks": make_masks(),
    }


_NC_CACHE = {}
LAST_RESULTS = None


def kernel(x, Wq, bq, Wk, bk, Wv, bv, Wp, bp):
    global LAST_RESULTS
    from concourse.bass_utils import run_bass_kernel_spmd

    x = np.asarray(x, np.float32)
    Wq, bq = np.asarray(Wq, np.float32), np.asarray(bq, np.float32)
    Wk, bk = np.asarray(Wk, np.float32), np.asarray(bk, np.float32)
    Wv, bv = np.asarray(Wv, np.float32), np.asarray(bv, np.float32)
    Wp, bp = np.asarray(Wp, np.float32), np.asarray(bp, np.float32)

    if "nc" not in _NC_CACHE:
        _NC_CACHE["nc"] = build_nc()
    nc = _NC_CACHE["nc"]

    in_maps = []
    for core in range(N_CORES):
        b, g = core // 2, core % 2
        in_maps.append(make_core_inputs(x, Wq, bq, Wk, bk, Wv, bv, Wp, b, g))

    res = run_bass_kernel_spmd(nc, in_maps, core_ids=list(range(N_CORES)))
    LAST_RESULTS = res

    out = np.empty((B, T, C), np.float32)
    for b in range(B):
        out[b] = res.results[2 * b]["out"] + res.results[2 * b + 1]["out"] + bp
    return out



# revision 52
# speedup vs baseline: 1.5159x; 1.5159x over previous
"""Causal self-attention Trainium2 Bass kernel.

Problem: B=4, T=2048, C=1024, NH=16, HD=64, fp32.
Sharding: 2D over 8 cores = 4 batches x 2 head-groups (8 heads each).
Each core computes, for its (batch b, head-group g):
    q/k/v = x[b] @ W{q,k,v}[rows_g].T + b{q,k,v}[rows_g]
    causal attention over its 8 heads
    partial_out = y_local @ Wp[:, cols_g].T        (host adds the two
    group partials per batch plus bp).

Schedule (per core): the four 512-wide query slices are processed as
one software-pipelined loop — projections for slice n (PE-heavy), then
causal attention for i-tile n (ACT-heavy exp), then the output
projection for those rows — so the Tile scheduler can fill each
engine's stalls with the neighbouring stage's work.

Projections run as compensated-fp8 DoubleRow matmuls: host splits
x = x8 + s8 and W' = 256*W = W8 + r8 (both fp8e4), and the kernel
accumulates x8@W8 + s8@W8 + x8@r8 (dropping the s8@r8 term, ~1e-3 of
the result).  DoubleRow contracts 256 rows per instruction at 0.5
cycles/row, so each 512-wide projection column block takes 12 matmuls
x 256 cycles instead of 8 x 512 — 25% fewer PE cycles and 4x fewer
weight DMA bytes... The 256x weight scale is folded downstream: q',k'
carry 256x (exp scale divides by 65536), v' carries 256x (the ones
column of vext is memset to 256 so the softmax denominator carries the
same scale and normalization cancels it).

Attention stays fp16:
    kT      : [128, T] per head-pair (head-dim on partitions, 2 heads
              stacked 64+64), persistent; qT is a transient [128, 512]
              per-pair tile for the current i-slice.
    S^T     : two K=64 row-tiled fp16 matmuls into one 2-bank PSUM tile
              [j=128, cols h0|h1].
    softmax : no max-subtraction; exp on ACT; causal masking via ONE
              in-place DVE multiply of the 128-wide diagonal slab with
              a [128,128] triangle mask (right of the slab is fully
              valid, left of it is a full off-diag block).
    y       : FLIPPED accumulation — lhsT = P [j, 128-query-subtile]
              (M=128, full PE array), rhs = v_h [j, 64], so a j-block
              costs 65 cycles per (head, subtile) vs 512 per head in
              the y^T orientation; the vext ones-column (=256) feeds a
              parallel N=1 matmul accumulating the softmax denominator
              per-PARTITION (psd).  All 8 (h,qb) sub-accumulations
              share one PSUM bank as a single zero-region group (start
              on the first write, stop on the last; per-element
              has_written bits zero each sub-region on first touch).
    norm    : reciprocal of psd + per-partition tensor_scalar muls,
              then PE transposes ([q,ch] -> [ch,q], fp16, identity
              rhs) rebuild y^T for the output projection.

The attention chain (S/exp/mask/y/norm/transpose) is emitted at raised
priority so the Tile scheduler treats projection and out-proj matmuls
as fillers for the ACT-bound chain; per-slice emission order is
attention(n), proj(n+1), out(n) so the next slice's q/k groups outrank
the out-projection at slice boundaries.  DMAs are batched (one per
weight tensor / x slice / output row-block) to minimize serialized
HWDGE holds, ordered so the first q-projection's operands land first.
"""

import numpy as np
import ml_dtypes

import concourse.bass as bass
import concourse.mybir as mybir
import concourse.tile as tile
from concourse import bacc
from concourse.tile_rust import add_dep_helper

B, T, C = 4, 2048, 1024
NH, HD = 16, 64
HPG = 8            # heads per group (per core)
NPAIR = HPG // 2   # head pairs per core
CL = HPG * HD      # 512 local channels
F32 = mybir.dt.float32
FP16 = mybir.dt.float16
FP8 = mybir.dt.float8e4
DR = mybir.MatmulPerfMode.DoubleRow
EXP = mybir.ActivationFunctionType.Exp
SCALE = 1.0 / np.sqrt(HD)
WS = 256.0         # fp8 weight pre-scale (folded: see docstring)
N_CORES = 8
NKD = 4            # DoubleRow contraction tiles (256 rows each)
PRIO_OFF = 800     # attention-chain priority boost (see emit_attn)


def attention_body(tc, outs, ins, t=T):
    nc = tc.nc
    nit = t // 512            # i-tiles (queries) == x slices
    njb = t // 128            # j-blocks (keys)

    x8T, s8T = ins["x8T"], ins["s8T"]            # [C, t] fp8
    w8q, r8q = ins["w8q"], ins["r8q"]            # [C, CL] fp8 (x256)
    w8k, r8k = ins["w8k"], ins["r8k"]
    w8v, r8v = ins["w8v"], ins["r8v"]
    wpT = ins["wpT"]          # [CL, C] fp16
    bqk = ins["bqk"]          # [128, 2*NPAIR] f32 (x256)
    bvt = ins["bvt"]          # [128, CL] f32 (x256)
    masks = ins["masks"]      # [128, 128] fp16 triangle
    out = outs["out"]         # [t, C] f32

    dum = {}

    def _absorb(deps, first_mms):
        """Absorb multi-lane PSUM slot-recycle deps into dummy LDWEIGHTS
        ops (one per dep) so the group's first matmul keeps at most one
        sync-wait (the fused LDW+MM struct allows only one; the
        wait-elision pass only credits real engine instructions)."""
        deps = [d for d in deps if d is not None]
        for d in deps:
            ld = nc.tensor.ldweights(weights=dum["t"][0:1, 0:1])
            add_dep_helper(ld.ins, d.ins, reason="absorb slot release")
            for mm in first_mms:
                add_dep_helper(mm.ins, ld.ins, sync=False,
                               reason="order after absorber")

    def wtile(pool, name):
        w = pool.tile([128, NKD, 2, CL], FP8, tag=name, name=name)
        return w

    def wdma(w, src, split=1):
        """One (or `split` kk-chunked) DMA(s) filling the [128,NKD,2,*]
        weight layout from a [C, *] DRAM tensor."""
        step = NKD // split
        for c in range(split):
            nc.sync.dma_start(
                out=w[:, c * step:(c + 1) * step],
                in_=src[256 * c * step:256 * (c + 1) * step, :].rearrange(
                    "(kk i p) c -> p kk i c", kk=step, i=2))

    with tc.tile_pool(name="consts", bufs=1) as consts, \
         tc.tile_pool(name="xin", bufs=2) as xin:
        dum["t"] = consts.tile([1, 2], FP16, tag="dum", name="dum")
        nc.vector.memset(dum["t"], 0)
        nc.tensor.ldweights(weights=dum["t"][0:1, 0:1])  # prime dum dep

        # ---- DMAs in consumption order: slice-0 x first, then weights.
        # Slice-0 transfers are split in kk-halves so the first
        # projection matmuls can start ~3us earlier.
        xts, sts = [None] * nit, [None] * nit

        def xdma(n, split=1):
            xt = xin.tile([128, NKD, 2, 512], FP8, tag="x8", name="x8")
            st = xin.tile([128, NKD, 2, 512], FP8, tag="s8", name="s8")
            step = NKD // split
            for c in range(split):
                for dst, src in ((xt, x8T), (st, s8T)):
                    nc.sync.dma_start(
                        out=dst[:, c * step:(c + 1) * step],
                        in_=src[256 * c * step:256 * (c + 1) * step,
                                512 * n:512 * (n + 1)].rearrange(
                            "(kk i p) t -> p kk i t", kk=step, i=2))
            xts[n], sts[n] = xt, st

        xdma(0, split=2)
        wq_t = wtile(consts, "wq8")
        wdma(wq_t, w8q, split=2)
        rq_t = wtile(consts, "rq8")
        wdma(rq_t, r8q)
        bqk_t = consts.tile([128, 2 * NPAIR], F32, tag="bqk", name="bqk_t")
        nc.sync.dma_start(out=bqk_t, in_=bqk)
        wk_t = wtile(consts, "wk8")
        wdma(wk_t, w8k)
        rk_t = wtile(consts, "rk8")
        wdma(rk_t, r8k)
        wv_t = wtile(consts, "wv8")
        wdma(wv_t, w8v)
        rv_t = wtile(consts, "rv8")
        wdma(rv_t, r8v)
        bvt_t = consts.tile([128, CL], F32, tag="bvt", name="bvt_t")
        nc.sync.dma_start(out=bvt_t, in_=bvt)
        mks = consts.tile([128, 128], FP16, tag="mks", name="mks")
        nc.sync.dma_start(out=mks, in_=masks)
        idn = consts.tile([128, 128], FP16, tag="idn", name="idn")
        nc.sync.dma_start(out=idn, in_=ins["ident"])
        wp_t = consts.tile([128, NPAIR, C], FP16, tag="wp", name="wp")
        nc.sync.dma_start(
            out=wp_t, in_=wpT.rearrange("(p4 p) c -> p p4 c", p4=NPAIR))

        with tc.tile_pool(name="persist", bufs=1) as pers, \
             tc.tile_pool(name="qy", bufs=2) as qy, \
             tc.tile_pool(name="ptp", bufs=16) as ptp, \
             tc.tile_pool(name="sm", bufs=2) as sm, \
             tc.tile_pool(name="ps1", bufs=2, space="PSUM") as ps1, \
             tc.tile_pool(name="psS", bufs=2, space="PSUM") as psS, \
             tc.tile_pool(name="psY", bufs=1, space="PSUM") as psY, \
             tc.tile_pool(name="psX", bufs=1, space="PSUM") as psX:
            kT = [pers.tile([128, t], FP16, tag=f"kT{p}", name=f"kT{p}")
                  for p in range(NPAIR)]
            vext = [pers.tile([128, njb * 130], FP16, tag=f"vext{p}",
                              name=f"vext{p}") for p in range(NPAIR)]
            for p in range(NPAIR):
                ones_view = vext[p][:, :].rearrange(
                    "q (jt two d) -> q jt two d", jt=njb, two=2)[:, :, :, 64:65]
                nc.vector.memset(ones_view, WS)

            ps1_hist = []   # (evictor, last mm) per ps1 slot (bufs=2)
            psS_hist = []   # ([readers], last mm) per psS slot (bufs=2);
            #                 shared by attention S blocks and out-proj
            psY_hist = []   # ([norm insts], [last y mms]) per pair
            psD_hist = []   # ([recip], [last D mms]) per pair
            psT_hist = []   # ([evict copy], last transpose) per pair

            # PE p-state warm-up: dummy matmuls on a zeroed tile keep
            # the PE continuously busy through the startup DMA wait, so
            # the first real matmuls run at full clock (the cost model
            # halves matmul speed until 3us of continuous busy).
            warm = pers.tile([128, 128], FP16, tag="warm", name="warm")
            nc.vector.memset(warm, 0)
            wps = psX.tile([128, 264], F32, tag="psX", name="warmps")
            for _ in range(40):
                nc.tensor.matmul(wps[:, 0:128], lhsT=warm, rhs=warm,
                                 start=True, stop=True)

            def group(body_mms, evict_fn, hist=ps1_hist, dist=2):
                k = len(hist)
                prev = hist[k - dist] if k >= dist else None
                mms = body_mms()
                if prev is not None:
                    _absorb([prev[0], prev[1]], [mms[0]])
                ev = evict_fn()
                hist.append((ev, mms[-1]))

            qTs_all, yTs_all = {}, {}

            def emit_proj(n):
                xt, st = xts[n], sts[n]

                # ---- q/k projections (compensated fp8 DoubleRow);
                # residual terms last so groups can start before the
                # residual-weight DMAs land on the first slice ----
                qTs = []
                for p in range(NPAIR):
                    qp = qy.tile([128, 512], FP16, tag=f"qT{p}",
                                 name=f"qT{p}")
                    qTs.append(qp)
                qTs_all[n] = qTs
                for wt, rt, boff, dsts in ((wq_t, rq_t, 0, "q"),
                                           (wk_t, rk_t, NPAIR, "k")):
                    for p in range(NPAIR):
                        ps = ps1.tile([128, 512], F32, tag="ps1",
                                      name="ps1q")

                        def mk(ps=ps, wt=wt, rt=rt, p=p, xt=xt, st=st):
                            mms = []
                            terms = (
                                [(kk, wt, xt) for kk in range(NKD)]
                                + [(kk, wt, st) for kk in range(NKD)]
                                + [(kk, rt, xt) for kk in range(NKD)])
                            for j, (kk, lh, rh) in enumerate(terms):
                                mms.append(nc.tensor.matmul(
                                    ps,
                                    lhsT=lh[:, kk, :,
                                            128 * p:128 * (p + 1)],
                                    rhs=rh[:, kk],
                                    start=(j == 0),
                                    stop=(j == len(terms) - 1),
                                    perf_mode=DR))
                            return mms

                        if dsts == "q":
                            def ev(ps=ps, p=p, boff=boff, qTs=qTs):
                                return nc.vector.tensor_scalar_add(
                                    out=qTs[p], in0=ps,
                                    scalar1=bqk_t[:, boff + p:boff + p + 1])
                        else:
                            def ev(ps=ps, p=p, n=n, boff=boff):
                                return nc.vector.tensor_scalar_add(
                                    out=kT[p][:, 512 * n:512 * (n + 1)],
                                    in0=ps,
                                    scalar1=bqk_t[:, boff + p:boff + p + 1])
                        group(mk, ev)
                # ---- v projection (compensated fp8 DoubleRow) ----
                for tb in range(4):
                    jt = 4 * n + tb
                    ps = ps1.tile([128, CL], F32, tag="ps1", name="ps1v")

                    def mk(ps=ps, tb=tb, xt=xt, st=st):
                        mms = []
                        terms = ([(kk, xt, wv_t) for kk in range(NKD)]
                                 + [(kk, st, wv_t) for kk in range(NKD)]
                                 + [(kk, xt, rv_t) for kk in range(NKD)])
                        for j, (kk, lh, rh) in enumerate(terms):
                            mms.append(nc.tensor.matmul(
                                ps,
                                lhsT=lh[:, kk, :,
                                        128 * tb:128 * (tb + 1)],
                                rhs=rh[:, kk],
                                start=(j == 0),
                                stop=(j == len(terms) - 1),
                                perf_mode=DR))
                        return mms

                    def ev(ps=ps, jt=jt):
                        last = None
                        for p in range(NPAIR):
                            dst = vext[p][:, 130 * jt:130 * (jt + 1)
                                          ].rearrange(
                                "q (two d) -> q two d", two=2)[:, :, 0:64]
                            last = nc.vector.tensor_add(
                                out=dst,
                                in0=ps[:, 128 * p:128 * (p + 1)].rearrange(
                                    "q (two d) -> q two d", two=2),
                                in1=bvt_t[:, 128 * p:128 * (p + 1)
                                          ].rearrange(
                                    "q (two d) -> q two d", two=2))
                        return last

                    group(mk, ev)

            def emit_attn(n):
                # ---- attention for i-tile n ----
                # y runs "flipped": lhsT = P [j, 128-query-subtile]
                # (M=128, full array), rhs = v_h [j, 64] (+1 ones col as
                # a separate N=1 matmul into psD), so each j-block costs
                # 65 cycles per (head, subtile) instead of 512 per head.
                # The softmax denominator lands per-PARTITION, making
                # normalization two tiny tensor_scalar ops; y comes out
                # [q, ch] and is transposed back to [ch, q] with PE
                # transposes for the output projection.  The whole chain
                # is emitted at raised priority so it preempts
                # projection/out-proj filler work on the PE.
                it = n
                njb_i = 4 * it + 4
                qTs = qTs_all[n]
                yTs = []
                yTs_all[n] = yTs
                for p in range(NPAIR):
                  with tc.high_priority(offset=PRIO_OFF):
                    prevy = psY_hist[-1] if len(psY_hist) >= 1 else None
                    prevd = psD_hist[-1] if len(psD_hist) >= 1 else None
                    psy = psY.tile([128, 2, 4, 64], F32, tag="psY",
                                   name="psy")
                    psx = psX.tile([128, 264], F32, tag="psX", name="psx")
                    psd = psx[:, 0:8].rearrange("p (h q) -> p h q", h=2)
                    pst = psx[:, 8:264].bitcast(FP16).rearrange(
                        "p (qb q) -> p qb q", qb=4)
                    first_ymms, first_dmms = [], []
                    last_ymms, last_dmms = [], []
                    for m in range(njb_i):
                        dm = m - 4 * it
                        off = 128 * dm if dm >= 0 else 0
                        ks = len(psS_hist)
                        prevs = psS_hist[ks - 2] if ks >= 2 else None
                        pss = psS.tile([128, 1024], F32, tag="psS",
                                       name="pss")
                        smms = []
                        for h in range(2):
                            hb = 64 * h
                            smms.append(nc.tensor.matmul(
                                pss[:, 512 * h + off:512 * (h + 1)],
                                lhsT=kT[p][hb:hb + 64,
                                           128 * m:128 * (m + 1)],
                                rhs=qTs[p][hb:hb + 64, off:512],
                                start=True, stop=True))
                        if prevs is not None:
                            _absorb(list(prevs[0]) + [prevs[1]],
                                    [smms[0]])
                        pt = ptp.tile([128, 2, 512], FP16, tag="pt",
                                      name="pt")
                        if dm < 0:
                            ex = nc.scalar.activation(
                                out=pt,
                                in_=pss.rearrange("q (h w) -> q h w", h=2),
                                func=EXP, scale=float(SCALE / (WS * WS)))
                        else:
                            pss3 = pss.rearrange(
                                "q (h w) -> q h w", h=2)[:, :, off:512]
                            ex = nc.scalar.activation(
                                out=pt[:, :, off:512], in_=pss3, func=EXP,
                                scale=float(SCALE / (WS * WS)))
                            # causal mask: only the 128-wide diagonal
                            # slab needs masking; in-place multiply.
                            nc.vector.tensor_mul(
                                out=pt[:, :, off:off + 128],
                                in0=pt[:, :, off:off + 128],
                                in1=mks.unsqueeze(1).broadcast_to(
                                    [128, 2, 128]))
                        psS_hist.append(([ex], smms[-1]))
                        for h in range(2):
                            vcol = 130 * m + 65 * h
                            for qb in range(4):
                                if dm >= 0 and qb < dm:
                                    continue
                                # the psy bank holds all 8 (h,qb)
                                # sub-accumulations as ONE zero-region
                                # group: start only on the very first
                                # write to the bank, stop only on the
                                # last; per-element has_written bits
                                # zero each sub-region on first touch.
                                st = (m == 0 and h == 0 and qb == 0)
                                sp = (m == njb_i - 1 and h == 1
                                      and qb == 3)
                                ymm = nc.tensor.matmul(
                                    psy[:, h, qb, :],
                                    lhsT=pt[:, h,
                                            128 * qb:128 * (qb + 1)],
                                    rhs=vext[p][:, vcol:vcol + 64],
                                    start=st, stop=sp)
                                dmm = nc.tensor.matmul(
                                    psd[:, h, qb:qb + 1],
                                    lhsT=pt[:, h,
                                            128 * qb:128 * (qb + 1)],
                                    rhs=vext[p][:, vcol + 64:vcol + 65],
                                    start=st, stop=sp)
                                if st:
                                    first_ymms.append(ymm)
                                    first_dmms.append(dmm)
                                if sp:
                                    last_ymms.append(ymm)
                                    last_dmms.append(dmm)
                    if prevy is not None:
                        _absorb(list(prevy[0]) + list(prevy[1]),
                                first_ymms)
                    if prevd is not None:
                        deps = list(prevd[0]) + list(prevd[1])
                        if len(psT_hist) >= 1:
                            deps += list(psT_hist[-1][0])
                        _absorb(deps, [first_dmms[0]])
                    # normalization: per-partition reciprocal + scale
                    rcp = sm.tile([128, 8], F32, tag="rcp", name="rcp",
                                  bufs=2)
                    rc = nc.vector.reciprocal(out=rcp, in_=psd)
                    yn = sm.tile([128, 4, 128], FP16, tag="yn", name="yn",
                                 bufs=2)
                    norms = []
                    for h in range(2):
                        for qb in range(4):
                            norms.append(nc.vector.tensor_scalar_mul(
                                out=yn[:, qb, 64 * h:64 * (h + 1)],
                                in0=psy[:, h, qb, :],
                                scalar1=rcp[:, 4 * h + qb:
                                            4 * h + qb + 1]))
                    psY_hist.append((norms, last_ymms))
                    psD_hist.append(([rc], last_dmms))
                    # transpose [q, ch] -> [ch, q] for the out-proj
                    prevt = psT_hist[-1] if len(psT_hist) >= 1 else None
                    tmms = []
                    for qb in range(4):
                        tmms.append(nc.tensor.matmul(
                            out=pst[:, qb, :], lhsT=yn[:, qb, :],
                            rhs=idn, is_transpose=True,
                            start=(qb == 0), stop=(qb == 3)))
                    if prevt is not None:
                        _absorb(list(prevt[0]) + [prevt[1]], [tmms[0]])
                    yp = qy.tile([128, 512], FP16, tag=f"yT{p}",
                                 name=f"yT{p}")
                    yTs.append(yp)
                    tev = nc.vector.tensor_copy(out=yp, in_=pst)
                    psT_hist.append(([tev], tmms[-1]))

            def emit_out(n):
                # ---- output projection for slice n's rows, via the
                # ps1 ring; emitted AFTER proj(n+1) so next-slice q/k
                # outrank it at the boundary ----
                yTs = yTs_all[n]
                for tb in range(4):
                    ot = sm.tile([128, 1024], F32, tag="ot", name="ot",
                                 bufs=2)
                    for oh in range(2):
                        pso = ps1.tile([128, 512], F32, tag="ps1",
                                       name="pso")

                        def mk(pso=pso, tb=tb, oh=oh, yTs=yTs):
                            return [nc.tensor.matmul(
                                pso,
                                lhsT=yTs[p][:, 128 * tb:128 * (tb + 1)],
                                rhs=wp_t[:, p, 512 * oh:512 * (oh + 1)],
                                start=(p == 0), stop=(p == NPAIR - 1))
                                for p in range(NPAIR)]

                        if n == nit - 1 and oh == 1:
                            def ev(pso=pso, ot=ot, oh=oh):
                                return nc.scalar.copy(
                                    out=ot[:, 512 * oh:512 * (oh + 1)],
                                    in_=pso)
                        else:
                            def ev(pso=pso, ot=ot, oh=oh):
                                return nc.vector.tensor_copy(
                                    out=ot[:, 512 * oh:512 * (oh + 1)],
                                    in_=pso)

                        group(mk, ev)
                        if n == nit - 1:
                            nc.sync.dma_start(
                                out=out[512 * n + 128 * tb:
                                        512 * n + 128 * (tb + 1),
                                        512 * oh:512 * (oh + 1)],
                                in_=ot[:, 512 * oh:512 * (oh + 1)])
                    if n != nit - 1:
                        nc.sync.dma_start(
                            out=out[512 * n + 128 * tb:
                                    512 * n + 128 * (tb + 1), :],
                            in_=ot)

            emit_proj(0)
            for n in range(nit):
                if n + 1 < nit:
                    xdma(n + 1)
                emit_attn(n)
                if n + 1 < nit:
                    emit_proj(n + 1)
                if n == nit - 1:
                    # out(nit-2) deferred past attention(nit-1)'s
                    # emission: its ps1 slots no longer gate any later
                    # projection, and its matmuls become ready filler
                    # for the last slice's ACT-bound attention chain.
                    emit_out(nit - 2)
                if n != nit - 2:
                    emit_out(n)


def build_nc(t=T):
    nc = bacc.Bacc("TRN2", target_bir_lowering=False, debug=False)
    ins = {}
    for nm in ("x8T", "s8T"):
        ins[nm] = nc.dram_tensor(nm, [C, t], FP8, kind="ExternalInput").ap()
    for nm in ("w8q", "r8q", "w8k", "r8k", "w8v", "r8v"):
        ins[nm] = nc.dram_tensor(nm, [C, CL], FP8, kind="ExternalInput").ap()
    ins["wpT"] = nc.dram_tensor("wpT", [CL, C], FP16,
                                kind="ExternalInput").ap()
    ins["bqk"] = nc.dram_tensor("bqk", [128, 2 * NPAIR], F32,
                                kind="ExternalInput").ap()
    ins["bvt"] = nc.dram_tensor("bvt", [128, CL], F32,
                                kind="ExternalInput").ap()
    ins["masks"] = nc.dram_tensor("masks", [128, 128], FP16,
                                  kind="ExternalInput").ap()
    ins["ident"] = nc.dram_tensor("ident", [128, 128], FP16,
                                  kind="ExternalInput").ap()
    outs = {
        "out": nc.dram_tensor("out", [t, C], F32, kind="ExternalOutput").ap(),
    }
    with tile.TileContext(nc) as tc:
        attention_body(tc, outs, ins, t=t)
    nc.compile()
    return nc


def make_masks():
    """[128,128] lower-triangle multiplicative mask: mk[j, c] = 1 iff
    j <= c. Applied to the 128-wide diagonal slab of each diagonal
    j-block (columns right of the slab are fully causal-valid)."""
    return np.ascontiguousarray(
        (np.arange(128)[:, None] <= np.arange(128)[None, :]
         ).astype(np.float16))


E4 = ml_dtypes.float8_e4m3


def _q8(a):
    return np.clip(a, -240, 240).astype(E4)


def _split8(a):
    hi = _q8(a)
    lo = _q8(a - hi.astype(np.float32))
    return hi, lo


def make_core_inputs(xb_hi, xb_lo, Wq8, Wk8, Wv8, bq, bk, bv, Wp, g):
    """Host-side shard + layout prep for core (batch b, head-group g).
    xb_hi/lo: [C, T] fp8 split of x[b].T (shared across the two
    head-group cores of a batch). W*8: per-group (hi, lo) fp8 splits of
    256*W[rows_g].T, precomputed once."""
    rows = slice(CL * g, CL * (g + 1))
    bqk = np.concatenate([bq[rows].reshape(NPAIR, 128).T,
                          bk[rows].reshape(NPAIR, 128).T], axis=1)
    return {
        "x8T": xb_hi, "s8T": xb_lo,
        "w8q": Wq8[0], "r8q": Wq8[1],
        "w8k": Wk8[0], "r8k": Wk8[1],
        "w8v": Wv8[0], "r8v": Wv8[1],
        "wpT": np.ascontiguousarray(Wp[:, rows].T.astype(np.float16)),
        "bqk": np.ascontiguousarray(WS * bqk),
        "bvt": np.ascontiguousarray(
            WS * np.tile(bv[rows][None, :], (128, 1)).astype(np.float32)),
        "masks": make_masks(),
        "ident": np.eye(128, dtype=np.float16),
    }


_NC_CACHE = {}
LAST_RESULTS = None


def kernel(x, Wq, bq, Wk, bk, Wv, bv, Wp, bp):
    global LAST_RESULTS
    from concourse.bass_utils import run_bass_kernel_spmd

    x = np.asarray(x, np.float32)
    Wq, bq = np.asarray(Wq, np.float32), np.asarray(bq, np.float32)
    Wk, bk = np.asarray(Wk, np.float32), np.asarray(bk, np.float32)
    Wv, bv = np.asarray(Wv, np.float32), np.asarray(bv, np.float32)
    Wp, bp = np.asarray(Wp, np.float32), np.asarray(bp, np.float32)

    if "nc" not in _NC_CACHE:
        _NC_CACHE["nc"] = build_nc()
    nc = _NC_CACHE["nc"]

    xsplits = [_split8(np.ascontiguousarray(x[b].T)) for b in range(B)]
    wsplits = []
    for g in range(2):
        rows = slice(CL * g, CL * (g + 1))
        wsplits.append({
            nm: _split8(WS * np.ascontiguousarray(W[rows, :].T))
            for nm, W in (("q", Wq), ("k", Wk), ("v", Wv))})

    in_maps = []
    for core in range(N_CORES):
        b, g = core // 2, core % 2
        ws = wsplits[g]
        in_maps.append(make_core_inputs(
            xsplits[b][0], xsplits[b][1], ws["q"], ws["k"], ws["v"],
            bq, bk, bv, Wp, g))

    res = run_bass_kernel_spmd(nc, in_maps, core_ids=list(range(N_CORES)))
    LAST_RESULTS = res

    out = np.empty((B, T, C), np.float32)
    for b in range(B):
        out[b] = res.results[2 * b]["out"] + res.results[2 * b + 1]["out"] + bp
    return out


# revision 53
# speedup vs baseline: 1.5162x; 1.0002x over previous
"""Causal self-attention Trainium2 Bass kernel.

Problem: B=4, T=2048, C=1024, NH=16, HD=64, fp32.
Sharding: 2D over 8 cores = 4 batches x 2 head-groups (8 heads each).
Each core computes, for its (batch b, head-group g):
    q/k/v = x[b] @ W{q,k,v}[rows_g].T + b{q,k,v}[rows_g]
    causal attention over its 8 heads
    partial_out = y_local @ Wp[:, cols_g].T        (host adds the two
    group partials per batch plus bp).

Schedule (per core): the four 512-wide query slices are processed as
one software-pipelined loop — projections for slice n (PE-heavy), then
causal attention for i-tile n (ACT-heavy exp), then the output
projection for those rows — so the Tile scheduler can fill each
engine's stalls with the neighbouring stage's work.

Projections run as compensated-fp8 DoubleRow matmuls: host splits
x = x8 + s8 and W' = 256*W = W8 + r8 (both fp8e4), and the kernel
accumulates x8@W8 + s8@W8 + x8@r8 (dropping the s8@r8 term, ~1e-3 of
the result).  DoubleRow contracts 256 rows per instruction at 0.5
cycles/row, so each 512-wide projection column block takes 12 matmuls
x 256 cycles instead of 8 x 512 — 25% fewer PE cycles and 4x fewer
weight DMA bytes... The 256x weight scale is folded downstream: q',k'
carry 256x (exp scale divides by 65536), v' carries 256x (the ones
column of vext is memset to 256 so the softmax denominator carries the
same scale and normalization cancels it).

Attention stays fp16:
    kT      : [128, T] per head-pair (head-dim on partitions, 2 heads
              stacked 64+64), persistent; qT is a transient [128, 512]
              per-pair tile for the current i-slice.
    S^T     : two K=64 row-tiled fp16 matmuls into one 2-bank PSUM tile
              [j=128, cols h0|h1].
    softmax : no max-subtraction; exp on ACT; causal masking via ONE
              in-place DVE multiply of the 128-wide diagonal slab with
              a [128,128] triangle mask (right of the slab is fully
              valid, left of it is a full off-diag block).
    y       : FLIPPED accumulation — lhsT = P [j, 128-query-subtile]
              (M=128, full PE array), rhs = v_h [j, 64], so a j-block
              costs 65 cycles per (head, subtile) vs 512 per head in
              the y^T orientation; the vext ones-column (=256) feeds a
              parallel N=1 matmul accumulating the softmax denominator
              per-PARTITION (psd).  All 8 (h,qb) sub-accumulations
              share one PSUM bank as a single zero-region group (start
              on the first write, stop on the last; per-element
              has_written bits zero each sub-region on first touch).
    norm    : reciprocal of psd + per-partition tensor_scalar muls,
              then PE transposes ([q,ch] -> [ch,q], fp16, identity
              rhs) rebuild y^T for the output projection.

The attention chain (S/exp/mask/y/norm/transpose) is emitted at raised
priority so the Tile scheduler treats projection and out-proj matmuls
as fillers for the ACT-bound chain; per-slice emission order is
attention(n), proj(n+1), out(n) so the next slice's q/k groups outrank
the out-projection at slice boundaries.  DMAs are batched (one per
weight tensor / x slice / output row-block) to minimize serialized
HWDGE holds, ordered so the first q-projection's operands land first.
"""

import numpy as np
import ml_dtypes

import concourse.bass as bass
import concourse.mybir as mybir
import concourse.tile as tile
from concourse import bacc
from concourse.tile_rust import add_dep_helper

B, T, C = 4, 2048, 1024
NH, HD = 16, 64
HPG = 8            # heads per group (per core)
NPAIR = HPG // 2   # head pairs per core
CL = HPG * HD      # 512 local channels
F32 = mybir.dt.float32
FP16 = mybir.dt.float16
FP8 = mybir.dt.float8e4
DR = mybir.MatmulPerfMode.DoubleRow
EXP = mybir.ActivationFunctionType.Exp
SCALE = 1.0 / np.sqrt(HD)
WS = 256.0         # fp8 weight pre-scale (folded: see docstring)
N_CORES = 8
NKD = 4            # DoubleRow contraction tiles (256 rows each)
PRIO_OFF = 800     # attention-chain priority boost (see emit_attn)


def attention_body(tc, outs, ins, t=T):
    nc = tc.nc
    nit = t // 512            # i-tiles (queries) == x slices
    njb = t // 128            # j-blocks (keys)

    x8T, s8T = ins["x8T"], ins["s8T"]            # [C, t] fp8
    w8q, r8q = ins["w8q"], ins["r8q"]            # [C, CL] fp8 (x256)
    w8k, r8k = ins["w8k"], ins["r8k"]
    w8v, r8v = ins["w8v"], ins["r8v"]
    wpT = ins["wpT"]          # [CL, C] fp16
    bqk = ins["bqk"]          # [128, 2*NPAIR] f32 (x256)
    bvt = ins["bvt"]          # [128, CL] f32 (x256)
    masks = ins["masks"]      # [128, 128] fp16 triangle
    out = outs["out"]         # [t, C] f32

    dum = {}

    def _absorb(deps, first_mms):
        """Absorb multi-lane PSUM slot-recycle deps into dummy LDWEIGHTS
        ops (one per dep) so the group's first matmul keeps at most one
        sync-wait (the fused LDW+MM struct allows only one; the
        wait-elision pass only credits real engine instructions)."""
        deps = [d for d in deps if d is not None]
        for d in deps:
            ld = nc.tensor.ldweights(weights=dum["t"][0:1, 0:1])
            add_dep_helper(ld.ins, d.ins, reason="absorb slot release")
            for mm in first_mms:
                add_dep_helper(mm.ins, ld.ins, sync=False,
                               reason="order after absorber")

    def wtile(pool, name):
        w = pool.tile([128, NKD, 2, CL], FP8, tag=name, name=name)
        return w

    def wdma(w, src, split=1):
        """One (or `split` kk-chunked) DMA(s) filling the [128,NKD,2,*]
        weight layout from a [C, *] DRAM tensor."""
        step = NKD // split
        for c in range(split):
            nc.sync.dma_start(
                out=w[:, c * step:(c + 1) * step],
                in_=src[256 * c * step:256 * (c + 1) * step, :].rearrange(
                    "(kk i p) c -> p kk i c", kk=step, i=2))

    with tc.tile_pool(name="consts", bufs=1) as consts, \
         tc.tile_pool(name="xin", bufs=2) as xin:
        dum["t"] = consts.tile([1, 2], FP16, tag="dum", name="dum")
        nc.vector.memset(dum["t"], 0)
        nc.tensor.ldweights(weights=dum["t"][0:1, 0:1])  # prime dum dep

        # ---- DMAs in consumption order: slice-0 x first, then weights.
        # Slice-0 transfers are split in kk-halves so the first
        # projection matmuls can start ~3us earlier.
        xts, sts = [None] * nit, [None] * nit

        def xdma(n, split=1):
            xt = xin.tile([128, NKD, 2, 512], FP8, tag="x8", name="x8")
            st = xin.tile([128, NKD, 2, 512], FP8, tag="s8", name="s8")
            step = NKD // split
            for c in range(split):
                for dst, src in ((xt, x8T), (st, s8T)):
                    nc.sync.dma_start(
                        out=dst[:, c * step:(c + 1) * step],
                        in_=src[256 * c * step:256 * (c + 1) * step,
                                512 * n:512 * (n + 1)].rearrange(
                            "(kk i p) t -> p kk i t", kk=step, i=2))
            xts[n], sts[n] = xt, st

        xdma(0, split=2)
        wq_t = wtile(consts, "wq8")
        wdma(wq_t, w8q, split=2)
        rq_t = wtile(consts, "rq8")
        wdma(rq_t, r8q)
        bqk_t = consts.tile([128, 2 * NPAIR], F32, tag="bqk", name="bqk_t")
        nc.sync.dma_start(out=bqk_t, in_=bqk)
        wk_t = wtile(consts, "wk8")
        wdma(wk_t, w8k)
        rk_t = wtile(consts, "rk8")
        wdma(rk_t, r8k)
        wv_t = wtile(consts, "wv8")
        wdma(wv_t, w8v)
        rv_t = wtile(consts, "rv8")
        wdma(rv_t, r8v)
        bvt_t = consts.tile([128, CL], F32, tag="bvt", name="bvt_t")
        nc.sync.dma_start(out=bvt_t, in_=bvt)
        mks = consts.tile([128, 128], FP16, tag="mks", name="mks")
        nc.sync.dma_start(out=mks, in_=masks)
        idn = consts.tile([128, 128], FP16, tag="idn", name="idn")
        nc.sync.dma_start(out=idn, in_=ins["ident"])
        wp_t = consts.tile([128, NPAIR, C], FP16, tag="wp", name="wp")
        nc.sync.dma_start(
            out=wp_t, in_=wpT.rearrange("(p4 p) c -> p p4 c", p4=NPAIR))

        with tc.tile_pool(name="persist", bufs=1) as pers, \
             tc.tile_pool(name="qy", bufs=2) as qy, \
             tc.tile_pool(name="ptp", bufs=20) as ptp, \
             tc.tile_pool(name="sm", bufs=2) as sm, \
             tc.tile_pool(name="ps1", bufs=2, space="PSUM") as ps1, \
             tc.tile_pool(name="psS", bufs=2, space="PSUM") as psS, \
             tc.tile_pool(name="psY", bufs=1, space="PSUM") as psY, \
             tc.tile_pool(name="psX", bufs=1, space="PSUM") as psX:
            kT = [pers.tile([128, t], FP16, tag=f"kT{p}", name=f"kT{p}")
                  for p in range(NPAIR)]
            vext = [pers.tile([128, njb * 130], FP16, tag=f"vext{p}",
                              name=f"vext{p}") for p in range(NPAIR)]
            for p in range(NPAIR):
                ones_view = vext[p][:, :].rearrange(
                    "q (jt two d) -> q jt two d", jt=njb, two=2)[:, :, :, 64:65]
                nc.vector.memset(ones_view, WS)

            ps1_hist = []   # (evictor, last mm) per ps1 slot (bufs=2)
            psS_hist = []   # ([readers], last mm) per psS slot (bufs=2);
            #                 shared by attention S blocks and out-proj
            psY_hist = []   # ([norm insts], [last y mms]) per pair
            psD_hist = []   # ([recip], [last D mms]) per pair
            psT_hist = []   # ([evict copy], last transpose) per pair

            # PE p-state warm-up: dummy matmuls on a zeroed tile keep
            # the PE continuously busy through the startup DMA wait, so
            # the first real matmuls run at full clock (the cost model
            # halves matmul speed until 3us of continuous busy).
            warm = pers.tile([128, 128], FP16, tag="warm", name="warm")
            nc.vector.memset(warm, 0)
            wps = psX.tile([128, 264], F32, tag="psX", name="warmps")
            for _ in range(40):
                nc.tensor.matmul(wps[:, 0:128], lhsT=warm, rhs=warm,
                                 start=True, stop=True)

            def group(body_mms, evict_fn, hist=ps1_hist, dist=2):
                k = len(hist)
                prev = hist[k - dist] if k >= dist else None
                mms = body_mms()
                if prev is not None:
                    _absorb([prev[0], prev[1]], [mms[0]])
                ev = evict_fn()
                hist.append((ev, mms[-1]))

            qTs_all, yTs_all = {}, {}

            def emit_proj(n):
                xt, st = xts[n], sts[n]

                # ---- q/k projections (compensated fp8 DoubleRow);
                # residual terms last so groups can start before the
                # residual-weight DMAs land on the first slice ----
                qTs = []
                for p in range(NPAIR):
                    qp = qy.tile([128, 512], FP16, tag=f"qT{p}",
                                 name=f"qT{p}")
                    qTs.append(qp)
                qTs_all[n] = qTs
                for wt, rt, boff, dsts in ((wq_t, rq_t, 0, "q"),
                                           (wk_t, rk_t, NPAIR, "k")):
                    for p in range(NPAIR):
                        ps = ps1.tile([128, 512], F32, tag="ps1",
                                      name="ps1q")

                        def mk(ps=ps, wt=wt, rt=rt, p=p, xt=xt, st=st):
                            mms = []
                            terms = (
                                [(kk, wt, xt) for kk in range(NKD)]
                                + [(kk, wt, st) for kk in range(NKD)]
                                + [(kk, rt, xt) for kk in range(NKD)])
                            for j, (kk, lh, rh) in enumerate(terms):
                                mms.append(nc.tensor.matmul(
                                    ps,
                                    lhsT=lh[:, kk, :,
                                            128 * p:128 * (p + 1)],
                                    rhs=rh[:, kk],
                                    start=(j == 0),
                                    stop=(j == len(terms) - 1),
                                    perf_mode=DR))
                            return mms

                        if dsts == "q":
                            def ev(ps=ps, p=p, boff=boff, qTs=qTs):
                                return nc.vector.tensor_scalar_add(
                                    out=qTs[p], in0=ps,
                                    scalar1=bqk_t[:, boff + p:boff + p + 1])
                        else:
                            def ev(ps=ps, p=p, n=n, boff=boff):
                                return nc.vector.tensor_scalar_add(
                                    out=kT[p][:, 512 * n:512 * (n + 1)],
                                    in0=ps,
                                    scalar1=bqk_t[:, boff + p:boff + p + 1])
                        group(mk, ev)
                # ---- v projection (compensated fp8 DoubleRow) ----
                for tb in range(4):
                    jt = 4 * n + tb
                    ps = ps1.tile([128, CL], F32, tag="ps1", name="ps1v")

                    def mk(ps=ps, tb=tb, xt=xt, st=st):
                        mms = []
                        terms = ([(kk, xt, wv_t) for kk in range(NKD)]
                                 + [(kk, st, wv_t) for kk in range(NKD)]
                                 + [(kk, xt, rv_t) for kk in range(NKD)])
                        for j, (kk, lh, rh) in enumerate(terms):
                            mms.append(nc.tensor.matmul(
                                ps,
                                lhsT=lh[:, kk, :,
                                        128 * tb:128 * (tb + 1)],
                                rhs=rh[:, kk],
                                start=(j == 0),
                                stop=(j == len(terms) - 1),
                                perf_mode=DR))
                        return mms

                    def ev(ps=ps, jt=jt):
                        last = None
                        for p in range(NPAIR):
                            dst = vext[p][:, 130 * jt:130 * (jt + 1)
                                          ].rearrange(
                                "q (two d) -> q two d", two=2)[:, :, 0:64]
                            last = nc.vector.tensor_add(
                                out=dst,
                                in0=ps[:, 128 * p:128 * (p + 1)].rearrange(
                                    "q (two d) -> q two d", two=2),
                                in1=bvt_t[:, 128 * p:128 * (p + 1)
                                          ].rearrange(
                                    "q (two d) -> q two d", two=2))
                        return last

                    group(mk, ev)

            def emit_attn(n):
                # ---- attention for i-tile n ----
                # y runs "flipped": lhsT = P [j, 128-query-subtile]
                # (M=128, full array), rhs = v_h [j, 64] (+1 ones col as
                # a separate N=1 matmul into psD), so each j-block costs
                # 65 cycles per (head, subtile) instead of 512 per head.
                # The softmax denominator lands per-PARTITION, making
                # normalization two tiny tensor_scalar ops; y comes out
                # [q, ch] and is transposed back to [ch, q] with PE
                # transposes for the output projection.  The whole chain
                # is emitted at raised priority so it preempts
                # projection/out-proj filler work on the PE.
                it = n
                njb_i = 4 * it + 4
                qTs = qTs_all[n]
                yTs = []
                yTs_all[n] = yTs
                for p in range(NPAIR):
                  with tc.high_priority(offset=PRIO_OFF):
                    prevy = psY_hist[-1] if len(psY_hist) >= 1 else None
                    prevd = psD_hist[-1] if len(psD_hist) >= 1 else None
                    psy = psY.tile([128, 2, 4, 64], F32, tag="psY",
                                   name="psy")
                    psx = psX.tile([128, 264], F32, tag="psX", name="psx")
                    psd = psx[:, 0:8].rearrange("p (h q) -> p h q", h=2)
                    pst = psx[:, 8:264].bitcast(FP16).rearrange(
                        "p (qb q) -> p qb q", qb=4)
                    first_ymms, first_dmms = [], []
                    last_ymms, last_dmms = [], []
                    for m in range(njb_i):
                        dm = m - 4 * it
                        off = 128 * dm if dm >= 0 else 0
                        ks = len(psS_hist)
                        prevs = psS_hist[ks - 2] if ks >= 2 else None
                        pss = psS.tile([128, 1024], F32, tag="psS",
                                       name="pss")
                        smms = []
                        for h in range(2):
                            hb = 64 * h
                            smms.append(nc.tensor.matmul(
                                pss[:, 512 * h + off:512 * (h + 1)],
                                lhsT=kT[p][hb:hb + 64,
                                           128 * m:128 * (m + 1)],
                                rhs=qTs[p][hb:hb + 64, off:512],
                                start=True, stop=True))
                        if prevs is not None:
                            _absorb(list(prevs[0]) + [prevs[1]],
                                    [smms[0]])
                        pt = ptp.tile([128, 2, 512], FP16, tag="pt",
                                      name="pt")
                        if dm < 0:
                            ex = nc.scalar.activation(
                                out=pt,
                                in_=pss.rearrange("q (h w) -> q h w", h=2),
                                func=EXP, scale=float(SCALE / (WS * WS)))
                        else:
                            pss3 = pss.rearrange(
                                "q (h w) -> q h w", h=2)[:, :, off:512]
                            ex = nc.scalar.activation(
                                out=pt[:, :, off:512], in_=pss3, func=EXP,
                                scale=float(SCALE / (WS * WS)))
                            # causal mask: only the 128-wide diagonal
                            # slab needs masking; in-place multiply.
                            nc.vector.tensor_mul(
                                out=pt[:, :, off:off + 128],
                                in0=pt[:, :, off:off + 128],
                                in1=mks.unsqueeze(1).broadcast_to(
                                    [128, 2, 128]))
                        psS_hist.append(([ex], smms[-1]))
                        for h in range(2):
                            vcol = 130 * m + 65 * h
                            for qb in range(4):
                                if dm >= 0 and qb < dm:
                                    continue
                                # the psy bank holds all 8 (h,qb)
                                # sub-accumulations as ONE zero-region
                                # group: start only on the very first
                                # write to the bank, stop only on the
                                # last; per-element has_written bits
                                # zero each sub-region on first touch.
                                st = (m == 0 and h == 0 and qb == 0)
                                sp = (m == njb_i - 1 and h == 1
                                      and qb == 3)
                                ymm = nc.tensor.matmul(
                                    psy[:, h, qb, :],
                                    lhsT=pt[:, h,
                                            128 * qb:128 * (qb + 1)],
                                    rhs=vext[p][:, vcol:vcol + 64],
                                    start=st, stop=sp)
                                dmm = nc.tensor.matmul(
                                    psd[:, h, qb:qb + 1],
                                    lhsT=pt[:, h,
                                            128 * qb:128 * (qb + 1)],
                                    rhs=vext[p][:, vcol + 64:vcol + 65],
                                    start=st, stop=sp)
                                if st:
                                    first_ymms.append(ymm)
                                    first_dmms.append(dmm)
                                if sp:
                                    last_ymms.append(ymm)
                                    last_dmms.append(dmm)
                    if prevy is not None:
                        _absorb(list(prevy[0]) + list(prevy[1]),
                                first_ymms)
                    if prevd is not None:
                        deps = list(prevd[0]) + list(prevd[1])
                        if len(psT_hist) >= 1:
                            deps += list(psT_hist[-1][0])
                        _absorb(deps, [first_dmms[0]])
                    # normalization: per-partition reciprocal + scale
                    rcp = sm.tile([128, 8], F32, tag="rcp", name="rcp",
                                  bufs=2)
                    rc = nc.vector.reciprocal(out=rcp, in_=psd)
                    yn = sm.tile([128, 4, 128], FP16, tag="yn", name="yn",
                                 bufs=2)
                    norms = []
                    for h in range(2):
                        for qb in range(4):
                            norms.append(nc.vector.tensor_scalar_mul(
                                out=yn[:, qb, 64 * h:64 * (h + 1)],
                                in0=psy[:, h, qb, :],
                                scalar1=rcp[:, 4 * h + qb:
                                            4 * h + qb + 1]))
                    psY_hist.append((norms, last_ymms))
                    psD_hist.append(([rc], last_dmms))
                    # transpose [q, ch] -> [ch, q] for the out-proj
                    prevt = psT_hist[-1] if len(psT_hist) >= 1 else None
                    tmms = []
                    for qb in range(4):
                        tmms.append(nc.tensor.matmul(
                            out=pst[:, qb, :], lhsT=yn[:, qb, :],
                            rhs=idn, is_transpose=True,
                            start=(qb == 0), stop=(qb == 3)))
                    if prevt is not None:
                        _absorb(list(prevt[0]) + [prevt[1]], [tmms[0]])
                    yp = qy.tile([128, 512], FP16, tag=f"yT{p}",
                                 name=f"yT{p}")
                    yTs.append(yp)
                    tev = nc.vector.tensor_copy(out=yp, in_=pst)
                    psT_hist.append(([tev], tmms[-1]))

            def emit_out(n):
                # ---- output projection for slice n's rows, via the
                # ps1 ring; emitted AFTER proj(n+1) so next-slice q/k
                # outrank it at the boundary ----
                yTs = yTs_all[n]
                for tb in range(4):
                    ot = sm.tile([128, 1024], F32, tag="ot", name="ot",
                                 bufs=3)
                    for oh in range(2):
                        pso = ps1.tile([128, 512], F32, tag="ps1",
                                       name="pso")

                        def mk(pso=pso, tb=tb, oh=oh, yTs=yTs):
                            return [nc.tensor.matmul(
                                pso,
                                lhsT=yTs[p][:, 128 * tb:128 * (tb + 1)],
                                rhs=wp_t[:, p, 512 * oh:512 * (oh + 1)],
                                start=(p == 0), stop=(p == NPAIR - 1))
                                for p in range(NPAIR)]

                        if n == nit - 1 and oh == 1:
                            def ev(pso=pso, ot=ot, oh=oh):
                                return nc.scalar.copy(
                                    out=ot[:, 512 * oh:512 * (oh + 1)],
                                    in_=pso)
                        else:
                            def ev(pso=pso, ot=ot, oh=oh):
                                return nc.vector.tensor_copy(
                                    out=ot[:, 512 * oh:512 * (oh + 1)],
                                    in_=pso)

                        group(mk, ev)
                        if n == nit - 1:
                            nc.sync.dma_start(
                                out=out[512 * n + 128 * tb:
                                        512 * n + 128 * (tb + 1),
                                        512 * oh:512 * (oh + 1)],
                                in_=ot[:, 512 * oh:512 * (oh + 1)])
                    if n != nit - 1:
                        nc.sync.dma_start(
                            out=out[512 * n + 128 * tb:
                                    512 * n + 128 * (tb + 1), :],
                            in_=ot)

            emit_proj(0)
            for n in range(nit):
                if n + 1 < nit:
                    xdma(n + 1)
                emit_attn(n)
                if n + 1 < nit:
                    emit_proj(n + 1)
                if n == nit - 1:
                    # out(nit-2) deferred past attention(nit-1)'s
                    # emission: its ps1 slots no longer gate any later
                    # projection, and its matmuls become ready filler
                    # for the last slice's ACT-bound attention chain.
                    emit_out(nit - 2)
                if n != nit - 2:
                    emit_out(n)


def build_nc(t=T):
    nc = bacc.Bacc("TRN2", target_bir_lowering=False, debug=False)
    ins = {}
    for nm in ("x8T", "s8T"):
        ins[nm] = nc.dram_tensor(nm, [C, t], FP8, kind="ExternalInput").ap()
    for nm in ("w8q", "r8q", "w8k", "r8k", "w8v", "r8v"):
        ins[nm] = nc.dram_tensor(nm, [C, CL], FP8, kind="ExternalInput").ap()
    ins["wpT"] = nc.dram_tensor("wpT", [CL, C], FP16,
                                kind="ExternalInput").ap()
    ins["bqk"] = nc.dram_tensor("bqk", [128, 2 * NPAIR], F32,
                                kind="ExternalInput").ap()
    ins["bvt"] = nc.dram_tensor("bvt", [128, CL], F32,
                                kind="ExternalInput").ap()
    ins["masks"] = nc.dram_tensor("masks", [128, 128], FP16,
                                  kind="ExternalInput").ap()
    ins["ident"] = nc.dram_tensor("ident", [128, 128], FP16,
                                  kind="ExternalInput").ap()
    outs = {
        "out": nc.dram_tensor("out", [t, C], F32, kind="ExternalOutput").ap(),
    }
    with tile.TileContext(nc) as tc:
        attention_body(tc, outs, ins, t=t)
    nc.compile()
    return nc


def make_masks():
    """[128,128] lower-triangle multiplicative mask: mk[j, c] = 1 iff
    j <= c. Applied to the 128-wide diagonal slab of each diagonal
    j-block (columns right of the slab are fully causal-valid)."""
    return np.ascontiguousarray(
        (np.arange(128)[:, None] <= np.arange(128)[None, :]
         ).astype(np.float16))


E4 = ml_dtypes.float8_e4m3


def _q8(a):
    return np.clip(a, -240, 240).astype(E4)


def _split8(a):
    hi = _q8(a)
    lo = _q8(a - hi.astype(np.float32))
    return hi, lo


def make_core_inputs(xb_hi, xb_lo, Wq8, Wk8, Wv8, bq, bk, bv, Wp, g):
    """Host-side shard + layout prep for core (batch b, head-group g).
    xb_hi/lo: [C, T] fp8 split of x[b].T (shared across the two
    head-group cores of a batch). W*8: per-group (hi, lo) fp8 splits of
    256*W[rows_g].T, precomputed once."""
    rows = slice(CL * g, CL * (g + 1))
    bqk = np.concatenate([bq[rows].reshape(NPAIR, 128).T,
                          bk[rows].reshape(NPAIR, 128).T], axis=1)
    return {
        "x8T": xb_hi, "s8T": xb_lo,
        "w8q": Wq8[0], "r8q": Wq8[1],
        "w8k": Wk8[0], "r8k": Wk8[1],
        "w8v": Wv8[0], "r8v": Wv8[1],
        "wpT": np.ascontiguousarray(Wp[:, rows].T.astype(np.float16)),
        "bqk": np.ascontiguousarray(WS * bqk),
        "bvt": np.ascontiguousarray(
            WS * np.tile(bv[rows][None, :], (128, 1)).astype(np.float32)),
        "masks": make_masks(),
        "ident": np.eye(128, dtype=np.float16),
    }


_NC_CACHE = {}
LAST_RESULTS = None


def kernel(x, Wq, bq, Wk, bk, Wv, bv, Wp, bp):
    global LAST_RESULTS
    from concourse.bass_utils import run_bass_kernel_spmd

    x = np.asarray(x, np.float32)
    Wq, bq = np.asarray(Wq, np.float32), np.asarray(bq, np.float32)
    Wk, bk = np.asarray(Wk, np.float32), np.asarray(bk, np.float32)
    Wv, bv = np.asarray(Wv, np.float32), np.asarray(bv, np.float32)
    Wp, bp = np.asarray(Wp, np.float32), np.asarray(bp, np.float32)

    if "nc" not in _NC_CACHE:
        _NC_CACHE["nc"] = build_nc()
    nc = _NC_CACHE["nc"]

    xsplits = [_split8(np.ascontiguousarray(x[b].T)) for b in range(B)]
    wsplits = []
    for g in range(2):
        rows = slice(CL * g, CL * (g + 1))
        wsplits.append({
            nm: _split8(WS * np.ascontiguousarray(W[rows, :].T))
            for nm, W in (("q", Wq), ("k", Wk), ("v", Wv))})

    in_maps = []
    for core in range(N_CORES):
        b, g = core // 2, core % 2
        ws = wsplits[g]
        in_maps.append(make_core_inputs(
            xsplits[b][0], xsplits[b][1], ws["q"], ws["k"], ws["v"],
            bq, bk, bv, Wp, g))

    res = run_bass_kernel_spmd(nc, in_maps, core_ids=list(range(N_CORES)))
    LAST_RESULTS = res

    out = np.empty((B, T, C), np.float32)
    for b in range(B):
        out[b] = res.results[2 * b]["out"] + res.results[2 * b + 1]["out"] + bp
    return out
